# revision 1
# baseline (speedup 1.0000x reference)
"""DRew-GNN stage (two masked GCN branches + softmax mix + residual) on 8 trn2 cores.

Strategy (graph/data parallel, dst-sharded):
  - core c owns dst nodes [c*6250, (c+1)*6250)
  - algebraic identity: segsum(coef * (x@W)[src]) == segsum(coef * x[src]) @ W,
    so all scatter/gather happens in x-space; each core ends with tiny matmuls.
  - per 128-edge tile: SWDGE dma_gather of x[src] rows (512B each) + a
    "valued one-hot" (DVE tensor_scalar: is_equal(iota, dstlocal) * rsqrt(degS*degD))
    then PE matmul  acc^T[f, dst] += X_tile^T(stationary) @ onehot(moving), fp32r.
  - self loops are synthetic diagonal tiles using the in-SBUF x_own slice.
  - finalize per 256-node window: acc -> @ (a_k * W_k) -> +bias -> relu ->
    PE transpose -> + residual -> DMA out.
  - host does integer-only index work (routing, bincount degrees, padding,
    descriptor layouts); every float op (softmax, rsqrt, scaling, matmul, relu)
    runs on device.
"""
import os
import sys

sys.path.insert(0, "/opt/trn_rl_repo")

import numpy as np

N = 50000
D = 128
NCORES = 8
DC = N // NCORES          # 6250 nodes per core
SUP = 512                 # super-chunk of dst nodes (PSUM acc width, 1 bank)
NSUP = (DC + SUP - 1) // SUP
NQ = SUP // 256           # sub-windows per super
SUB = 256                 # finalize window / one-hot width
TILE = 128
LO = 32768                # int16 gather index limit split
GB_TILES = 6              # max tiles per dma_gather call (SWDGE ring limit: >=16 crashes)
ACC_BUFS = 4              # two branches x double-buffered supers


def set_config(sup=None, gb=None, acc_bufs=None):
    global SUP, NSUP, NQ, GB_TILES, ACC_BUFS
    if sup is not None:
        SUP = sup
        NSUP = (DC + SUP - 1) // SUP
        NQ = SUP // 256
    if gb is not None:
        GB_TILES = gb
    if acc_bufs is not None:
        ACC_BUFS = acc_bufs


def _valid_half(s, q, h):
    return s * SUP + q * SUB + h * 128 < DC


def _valid_sub(s, q):
    return s * SUP + q * SUB < DC


def _sub_rows(s, q, h):
    """number of valid node rows in (super s, sub q, half h)"""
    base = s * SUP + q * SUB + h * 128
    return max(0, min(128, DC - base))


def _build_schedule(edge_index, edge_attr):
    """Integer-only host preprocessing. Returns (sched, per_core_arrays)."""
    src = np.asarray(edge_index[0]).astype(np.int64)
    dst = np.asarray(edge_index[1]).astype(np.int64)
    attr = np.asarray(edge_attr).astype(np.int64)

    deg = np.stack([
        np.bincount(dst[attr == 1], minlength=N),
        np.bincount(dst[attr == 2], minlength=N),
    ]) + 1  # [2, N] int

    mask = (attr == 1) | (attr == 2)
    s_, d_, k_ = src[mask], dst[mask], attr[mask] - 1  # k_ in {0,1}
    core = d_ // DC
    rel = d_ % DC
    sup = rel // SUP
    q_ = (rel % SUP) // SUB
    half = (s_ >= LO).astype(np.int64)

    key = ((((core * NSUP + sup) * 2 + half) * 2 + k_) * NQ + q_)
    order = np.argsort(key, kind="stable")
    ks = key[order]
    NKEY = NCORES * NSUP * 2 * 2 * NQ
    starts = np.searchsorted(ks, np.arange(NKEY + 1))
    cnt = (starts[1:] - starts[:-1]).reshape(NCORES, NSUP, 2, 2, NQ)
    T_run = -(-cnt.max(axis=0) // TILE)  # [NSUP, 2, 2, 4] shared tile counts

    # ---- uniform tile list ----
    # stream position assignment (gather buffer layout) is per (s, half),
    # but tile PROCESSING order is group-major per (s, k, q): all matmuls of
    # one PSUM accumulation group must be consecutive (2KB zero-region rule).
    tiles = []      # dicts: kind g/self, s, half, k, q, t(stream pos), h
    run_base = np.zeros((NSUP, 2, 2, NQ), np.int64)  # tile offset of run in (s,half) stream
    Tsh = np.zeros((NSUP, 2), np.int64)
    for s in range(NSUP):
        for hf in (0, 1):
            off = 0
            for k in (0, 1):
                for q in range(NQ):
                    run_base[s, hf, k, q] = off
                    off += int(T_run[s, hf, k, q])
            Tsh[s, hf] = off
        for k in (0, 1):
            for q in range(NQ):
                for hf in (0, 1):
                    base = run_base[s, hf, k, q]
                    for j in range(int(T_run[s, hf, k, q])):
                        tiles.append(dict(kind="g", s=s, half=hf, k=k, q=q,
                                          t=int(base) + j))
                for h in (0, 1):
                    if _valid_half(s, q, h):
                        tiles.append(dict(kind="self", s=s, k=k, q=q, h=h))

    # start/stop flags per accumulation group (s, k, q)
    first_seen, last_seen = {}, {}
    for i, t in enumerate(tiles):
        g = (t["s"], t["k"], t["q"])
        if g not in first_seen:
            first_seen[g] = i
        last_seen[g] = i
    for i, t in enumerate(tiles):
        g = (t["s"], t["k"], t["q"])
        t["start"] = first_seen[g] == i
        t["stop"] = last_seen[g] == i

    # gather batches per (s, half): list of (t0, nt)
    batches = {}
    for s in range(NSUP):
        for hf in (0, 1):
            bl = []
            t0 = 0
            while t0 < Tsh[s, hf]:
                nt = min(GB_TILES, int(Tsh[s, hf]) - t0)
                bl.append((t0, nt))
                t0 += nt
            batches[(s, hf)] = bl

    # map (s, half, t) -> global tile idx (meta column)
    gmap = {}
    for i, t in enumerate(tiles):
        if t["kind"] == "g":
            gmap[(t["s"], t["half"], t["t"])] = i
    Ttot = len(tiles)

    # ---- per-core data arrays ----
    per_core = []
    for c in range(NCORES):
        idx_arr = {}
        for s in range(NSUP):
            for hf in (0, 1):
                if Tsh[s, hf] > 0:
                    idx_arr[(s, hf)] = np.zeros((128, int(Tsh[s, hf]) * 8), np.int16)
        dl = np.full((128, Ttot), -1.0, np.float32)
        m1 = np.ones((128, Ttot), np.float32)

        for s in range(NSUP):
            for hf in (0, 1):
                for k in (0, 1):
                    for q in range(NQ):
                        kk = (((c * NSUP + s) * 2 + hf) * 2 + k) * NQ + q
                        e = order[starts[kk]:starts[kk + 1]]
                        n = len(e)
                        if n == 0:
                            continue
                        g = run_base[s, hf, k, q] * TILE + np.arange(n)
                        # idx layout: logical i -> [i%16 + 16*grp, i//16]
                        iv = (s_[e] - hf * LO).astype(np.int16)
                        ia = idx_arr[(s, hf)]
                        for grp in range(8):
                            ia[g % 16 + 16 * grp, g // 16] = iv
                        # meta
                        tloc = g // TILE
                        cols = np.array([gmap[(s, hf, int(t))] for t in tloc])
                        p = g % TILE
                        dl[p, cols] = (d_[e] % DC - s * SUP - q * SUB).astype(np.float32)
                        m1[p, cols] = (deg[k, s_[e]] * deg[k, d_[e]]).astype(np.float32)
        # self tiles
        for i, t in enumerate(tiles):
            if t["kind"] != "self":
                continue
            s, k, q, h = t["s"], t["k"], t["q"], t["h"]
            nrows = _sub_rows(s, q, h)
            p = np.arange(nrows)
            nodes = c * DC + s * SUP + q * SUB + h * 128 + p
            dl[p, i] = (h * 128 + p).astype(np.float32)
            m1[p, i] = (deg[k, nodes] ** 2).astype(np.float32)
        per_core.append(dict(idx=idx_arr, dl=dl, m1=m1))

    sched = dict(tiles=tiles, Tsh=Tsh, batches=batches, Ttot=Ttot, gmap=gmap)
    return sched, per_core


def _build_program(sched, sup_limit=None, skip_fin=False, ncores=NCORES, probe=(), bufs=None):
    from contextlib import ExitStack
    from concourse import bacc, mybir
    import concourse.tile as tile

    f32 = mybir.dt.float32
    f32r = mybir.dt.float32r
    i16 = mybir.dt.int16
    Alu = mybir.AluOpType
    Act = mybir.ActivationFunctionType

    tiles = sched["tiles"]
    Tsh = sched["Tsh"]
    batches = sched["batches"]
    Ttot = sched["Ttot"]

    bufs = {**dict(oh=8, r=4, ob=6, ac=6, xo=18, gb=14, idx=3), **(bufs or {})}
    nc = bacc.Bacc("TRN2", target_bir_lowering=False, debug=False,
                   num_devices=ncores)

    # DRAM I/O
    x_d = nc.dram_tensor("x", [N, D], f32, kind="ExternalInput").ap()
    xo_d = nc.dram_tensor("xown", [6272, D], f32, kind="ExternalInput").ap()
    w_d = [nc.dram_tensor(f"W{k+1}", [D, D], f32, kind="ExternalInput").ap()
           for k in (0, 1)]
    b_d = [nc.dram_tensor(f"b{k+1}c", [D, 1], f32, kind="ExternalInput").ap()
           for k in (0, 1)]
    al_d = nc.dram_tensor("alpha2", [1, 2], f32, kind="ExternalInput").ap()
    iota_d = nc.dram_tensor("iota", [128, SUB], f32, kind="ExternalInput").ap()
    iden_d = nc.dram_tensor("ident", [128, 128], f32, kind="ExternalInput").ap()
    ones_d = nc.dram_tensor("ones1", [1, 128], f32, kind="ExternalInput").ap()
    dl_d = nc.dram_tensor("mdl", [128, Ttot], f32, kind="ExternalInput").ap()
    m1_d = nc.dram_tensor("mm1", [128, Ttot], f32, kind="ExternalInput").ap()
    idx_d = {}
    for (s, hf), _bl in batches.items():
        if Tsh[s, hf] > 0:
            idx_d[(s, hf)] = nc.dram_tensor(
                f"idx_{s}_{hf}", [128, int(Tsh[s, hf]) * 8], i16,
                kind="ExternalInput").ap()
    out_d = nc.dram_tensor("out", [DC, D], f32, kind="ExternalOutput").ap()

    with tile.TileContext(nc) as tc, ExitStack() as ctx:
        const_p = ctx.enter_context(tc.tile_pool(name="const", bufs=1))
        meta_p = ctx.enter_context(tc.tile_pool(name="meta", bufs=1))
        idx_p = ctx.enter_context(tc.tile_pool(name="idx", bufs=bufs["idx"]))
        gb_p = ctx.enter_context(tc.tile_pool(name="gb", bufs=bufs["gb"]))
        oh_p = ctx.enter_context(tc.tile_pool(name="oh", bufs=bufs["oh"]))
        xo_p = ctx.enter_context(tc.tile_pool(name="xo", bufs=bufs["xo"]))
        ac_p = ctx.enter_context(tc.tile_pool(name="ac", bufs=bufs["ac"]))
        r_p = ctx.enter_context(tc.tile_pool(name="r", bufs=bufs["r"]))
        ob_p = ctx.enter_context(tc.tile_pool(name="ob", bufs=bufs["ob"]))
        acc_p = ctx.enter_context(tc.tile_pool(name="acc", bufs=ACC_BUFS, space="PSUM"))
        u_p = ctx.enter_context(tc.tile_pool(name="u", bufs=2, space="PSUM"))
        tp_p = ctx.enter_context(tc.tile_pool(name="tp", bufs=2, space="PSUM"))

        # ---------- prologue: constants, weights, softmax ----------
        iota_t = const_p.tile([128, SUB], f32)
        nc.sync.dma_start(iota_t[:], iota_d[:])
        iden_t = const_p.tile([128, 128], f32)
        nc.sync.dma_start(iden_t[:], iden_d[:])
        ones_t = const_p.tile([1, 128], f32)
        nc.sync.dma_start(ones_t[:], ones_d[:])
        w_t = []
        for k in (0, 1):
            wt = const_p.tile([128, 128], f32, tag=f"wraw{k}")
            nc.sync.dma_start(wt[:], w_d[k][:])
            w_t.append(wt)
        b_t = []
        for k in (0, 1):
            bt = const_p.tile([128, 1], f32, tag=f"braw{k}")
            nc.sync.dma_start(bt[:], b_d[k][:])
            b_t.append(bt)
        al_t = const_p.tile([1, 2], f32)
        nc.sync.dma_start(al_t[:], al_d[:])

        dl_t = meta_p.tile([128, Ttot], f32)
        nc.sync.dma_start(dl_t[:], dl_d[:])
        m1_t = meta_p.tile([128, Ttot], f32)
        nc.sync.dma_start(m1_t[:], m1_d[:])
        # values = 1/sqrt(m1)
        sq_t = meta_p.tile([128, Ttot], f32)
        nc.scalar.activation(sq_t[:], m1_t[:], Act.Sqrt)
        val_t = meta_p.tile([128, Ttot], f32)
        nc.vector.reciprocal(val_t[:], sq_t[:])

        # softmax(alpha) on device
        e_t = const_p.tile([1, 2], f32)
        nc.scalar.activation(e_t[:], al_t[:], Act.Exp)
        su_t = const_p.tile([1, 1], f32)
        nc.vector.tensor_tensor(su_t[:], e_t[:, 0:1], e_t[:, 1:2], Alu.add)
        rs_t = const_p.tile([1, 1], f32)
        nc.vector.reciprocal(rs_t[:], su_t[:])
        a_t = const_p.tile([1, 2], f32)
        nc.vector.tensor_scalar(a_t[:], e_t[:], rs_t[:], None, Alu.mult)
        # broadcast a over 128 partitions via rank-1 matmul
        abc_ps = u_p.tile([128, SUB], f32, tag="u")
        nc.tensor.matmul(abc_ps[:, 0:2], lhsT=ones_t[:], rhs=a_t[:],
                         start=True, stop=True)
        abc_t = const_p.tile([128, 2], f32)
        nc.vector.tensor_copy(abc_t[:], abc_ps[:, 0:2])
        # W'_k = a_k * W_k ;  bconst = a0*b1 + a1*b2
        wp_t = []
        for k in (0, 1):
            wp = const_p.tile([128, 128], f32, tag=f"wp{k}")
            nc.vector.tensor_scalar(wp[:].bitcast(f32r), w_t[k][:],
                                    abc_t[:, k:k + 1], None, Alu.mult)
            wp_t.append(wp)
        bc0 = const_p.tile([128, 1], f32, tag="btmp0")
        nc.vector.tensor_scalar(bc0[:], b_t[0][:], abc_t[:, 0:1], None, Alu.mult)
        bc1 = const_p.tile([128, 1], f32, tag="btmp1")
        nc.vector.tensor_scalar(bc1[:], b_t[1][:], abc_t[:, 1:2], None, Alu.mult)
        bconst = const_p.tile([128, 1], f32)
        nc.vector.tensor_tensor(bconst[:], bc0[:], bc1[:], Alu.add)

        # ---------- main loop over super-chunks ----------
        for s in range(NSUP if sup_limit is None else sup_limit):
            # gather batches for this super-chunk
            gbufs = {}  # (half, batch_index) -> (tile, t0, nt)
            for hf in (0, 1):
                if Tsh[s, hf] == 0:
                    continue
                it = idx_p.tile([128, int(Tsh[s, hf]) * 8], i16, tag="idx")
                nc.sync.dma_start(it[:], idx_d[(s, hf)][:])
                for bi, (t0, nt) in enumerate(batches[(s, hf)]):
                    gb = gb_p.tile([128, nt, 128], f32r, tag="gb")
                    src_ap = x_d[0:LO, :] if hf == 0 else x_d[LO:N, :]
                    if "no_gather" not in probe:
                        nc.gpsimd.dma_gather(gb[:], src_ap.bitcast(f32r),
                                             it[:, t0 * 8:(t0 + nt) * 8],
                                             nt * 128, nt * 128, 128)
                    gbufs[(hf, bi)] = (gb, t0, nt)

            # x_own tiles for this super-chunk (self matmuls + residual)
            xo_tiles = {}
            for q in range(NQ):
                for h in (0, 1):
                    if not _valid_half(s, q, h):
                        continue
                    xo = xo_p.tile([128, 128], f32r, tag="xo")
                    r0 = s * SUP + q * SUB + h * 128
                    nc.sync.dma_start(xo[:], xo_d[r0:r0 + 128, :].bitcast(f32r))
                    xo_tiles[(q, h)] = xo

            # PSUM accumulators [f, 1024] per branch
            accs = [acc_p.tile([128, SUP], f32, tag="acc", name=f"acc{s}_{_k}") for _k in (0, 1)]

            # edge + self tiles
            for i, t in enumerate(tiles):
                if t["s"] != s:
                    continue
                col = i
                if "no_oh" in probe:
                    oh = iota_t
                else:
                    oh = oh_p.tile([128, SUB], f32, tag="oh")
                    nc.vector.tensor_scalar(oh[:].bitcast(f32r), iota_t[:],
                                            dl_t[:, col:col + 1],
                                            val_t[:, col:col + 1],
                                            Alu.is_equal, Alu.mult)
                if t["kind"] == "g":
                    hf = t["half"]
                    bi = t["t"] // GB_TILES
                    gb, t0, nt = gbufs[(hf, bi)]
                    stat = (iden_t[:].bitcast(f32r) if "no_gather" in probe
                            else gb[:, t["t"] - t0, :])
                else:
                    stat = xo_tiles[(t["q"], t["h"])][:]
                q = t["q"]
                if "no_mm" not in probe:
                    nc.tensor.matmul(accs[t["k"]][:, q * SUB:(q + 1) * SUB],
                                     lhsT=stat, rhs=oh[:].bitcast(f32r),
                                     start=t["start"], stop=t["stop"])

            # finalize each 256-wide sub-window
            for q in range(NQ):
                if skip_fin or not _valid_sub(s, q):
                    continue
                u_ps = u_p.tile([128, SUB], f32, tag="u")
                for k in (0, 1):
                    ac = ac_p.tile([128, SUB], f32, tag="ac")
                    nc.scalar.activation(ac[:].bitcast(f32r),
                                         accs[k][:, q * SUB:(q + 1) * SUB],
                                         Act.Copy)
                    nc.tensor.matmul(u_ps[:], lhsT=wp_t[k][:].bitcast(f32r),
                                     rhs=ac[:].bitcast(f32r),
                                     start=(k == 0), stop=(k == 1))
                r_t = r_p.tile([128, SUB], f32, tag="r")
                nc.scalar.activation(r_t[:], u_ps[:], Act.Relu, bias=bconst[:])
                for h in (0, 1):
                    nrows = _sub_rows(s, q, h)
                    if nrows <= 0:
                        continue
                    tp = tp_p.tile([128, 128], f32, tag="tp")
                    nc.tensor.transpose(tp[:], r_t[:, h * 128:(h + 1) * 128],
                                        iden_t[:])
                    ob = ob_p.tile([128, 128], f32, tag="ob")
                    nc.vector.tensor_tensor(ob[:], tp[:],
                                            xo_tiles[(q, h)][:].bitcast(f32),
                                            Alu.add)
                    r0 = s * SUP + q * SUB + h * 128
                    nc.sync.dma_start(out_d[r0:r0 + nrows, :], ob[0:nrows, :])

    nc.compile()
    return nc


def _make_in_maps(x, W1, b1, W2, b2, alpha, sched, per_core):
    x = np.ascontiguousarray(np.asarray(x, np.float32))
    consts = dict(
        W1=np.asarray(W1, np.float32), W2=np.asarray(W2, np.float32),
        b1c=np.asarray(b1, np.float32).reshape(D, 1),
        b2c=np.asarray(b2, np.float32).reshape(D, 1),
        alpha2=np.asarray(alpha, np.float32).reshape(1, 2),
        iota=np.tile(np.arange(SUB, dtype=np.float32), (128, 1)),
        ident=np.eye(128, dtype=np.float32),
        ones1=np.ones((1, 128), np.float32),
    )
    in_maps = []
    for c in range(NCORES):
        m = dict(consts)
        m["x"] = x
        xop = np.zeros((6272, D), np.float32)
        xop[:DC] = x[c * DC:(c + 1) * DC]
        m["xown"] = xop
        m["mdl"] = per_core[c]["dl"]
        m["mm1"] = per_core[c]["m1"]
        for (s, hf), arr in per_core[c]["idx"].items():
            m[f"idx_{s}_{hf}"] = arr
        in_maps.append(m)
    return in_maps


def _run(inputs, trace=False):
    from concourse.bass_utils import run_bass_kernel_spmd

    sched, per_core = _build_schedule(inputs["edge_index"], inputs["edge_attr"])
    nc = _build_program(sched)
    in_maps = _make_in_maps(inputs["x"], inputs["W1"], inputs["b1"],
                            inputs["W2"], inputs["b2"], inputs["alpha"],
                            sched, per_core)
    res = run_bass_kernel_spmd(nc, in_maps, list(range(NCORES)), trace=trace)
    out = np.concatenate([res.results[c]["out"] for c in range(NCORES)], axis=0)
    return out.astype(np.float32), res


def kernel(x, edge_index, edge_attr, W1, b1, W2, b2, alpha):
    inputs = dict(x=x, edge_index=edge_index, edge_attr=edge_attr,
                  W1=W1, b1=b1, W2=W2, b2=b2, alpha=alpha)
    out, _ = _run(inputs, trace=False)
    return out



# revision 23
# speedup vs baseline: 1.9192x; 1.9192x over previous
"""DRew-GNN stage (two masked GCN branches + softmax mix + residual) on 8 trn2 cores.

Strategy (graph/data parallel, dst-sharded):
  - core c owns dst nodes [c*6250, (c+1)*6250)
  - algebraic identity: segsum(coef * (x@W)[src]) == segsum(coef * x[src]) @ W,
    so all scatter/gather happens in x-space; each core ends with tiny matmuls.
  - per 128-edge tile: SWDGE dma_gather of x[src] rows (512B each) + a
    "valued one-hot" in bf16 (DVE tensor_scalar 4x mode: is_equal(iota,
    dstlocal) * rsqrt(degS*degD)) then PE matmul
    acc^T[f, dst] += X_tile^T(stationary f32r) @ onehot(moving bf16).
  - self loops are synthetic diagonal tiles using the in-SBUF x_own slice.
  - finalize per 256-node window: acc -> @ (a_k * W_k) -> +bias -> relu ->
    PE transpose -> + residual -> DMA out (one DMA per window).
  - host does integer-only index work (routing, bincount degrees, padding,
    descriptor layouts); every float op (softmax, rsqrt, scaling, matmul, relu)
    runs on device.
"""
import os
import sys

sys.path.insert(0, "/opt/trn_rl_repo")

import numpy as np

N = 50000
D = 128
NCORES = 8
DC = N // NCORES          # 6250 nodes per core
SUP = 512                 # super-chunk of dst nodes (PSUM acc width, 1 bank)
NSUP = (DC + SUP - 1) // SUP
NQ = SUP // 256           # sub-windows per super
SUB = 256                 # finalize window / one-hot width
TILE = 128
LO = 32768                # int16 gather index limit split
GB_TILES = 11             # max tiles per dma_gather call (HW crashes at >=12)
SCRATCH = 16384           # SWDGE dynamic dma scratch bytes
POOL_OH_FRAC = 0.0        # fraction of gather-tile one-hots issued on Pool
SELF_OH_POOL = False      # build self-tile one-hots on Pool engine
RES_POOL = False          # residual add on Pool (INVALID: GPSIMD cannot access PSUM)
BF16_GB = True            # convert gathered tiles to bf16; bf16 one-hots
CONV_DVE_MOD = 3          # every Nth gb-batch conversion on DVE (0=never)
XO_CONV_DVE = False       # xo conversion on DVE instead of Act
AC_DVE = False            # ac PSUM->SBUF copy on DVE instead of Act
ACC_BUFS = 4              # two branches x double-buffered supers
NWIN = (DC + SUB - 1) // SUB   # 25 output windows per core
XPS = SUP // 128          # xo half-tiles per super
NXO = NSUP * XPS          # xo3 tiles (padded so every super loads XPS)


def set_config(sup=None, gb=None, acc_bufs=None, scratch=None):
    global SUP, NSUP, NQ, GB_TILES, ACC_BUFS, SCRATCH, XPS, NXO
    if sup is not None:
        SUP = sup
        NSUP = (DC + SUP - 1) // SUP
        NQ = SUP // 256
        XPS = SUP // 128
        NXO = NSUP * XPS
    if gb is not None:
        GB_TILES = gb
    if acc_bufs is not None:
        ACC_BUFS = acc_bufs
    if scratch is not None:
        SCRATCH = scratch


def _valid_half(s, q, h):
    return s * SUP + q * SUB + h * 128 < DC


def _valid_sub(s, q):
    return s * SUP + q * SUB < DC


def _sub_rows(s, q, h):
    """number of valid node rows in (super s, sub q, half h)"""
    base = s * SUP + q * SUB + h * 128
    return max(0, min(128, DC - base))


def _balanced_batches(T, gb):
    """split T tiles into ceil(T/gb) near-equal batches -> list of (t0, nt)"""
    if T <= 0:
        return []
    nb = -(-T // gb)
    base = T // nb
    rem = T % nb
    out = []
    t0 = 0
    for i in range(nb):
        nt = base + (1 if i < rem else 0)
        out.append((t0, nt))
        t0 += nt
    return out


def _balance_nodes(d_, k_, half):
    """Assign nodes to (core, window-position) slots so per-run edge counts
    are balanced across cores (shrinks shared tile-count padding).
    Integer-only. Returns (core_of, rel_of, pos2node[NCORES, DC])."""
    cj = half * 2 + k_
    cnt = np.zeros((N, 4), np.int64)
    np.add.at(cnt, (d_, cj), 1)
    tot = cnt.sum(1)
    order = np.argsort(-tot, kind="stable")

    NB = NCORES * NWIN  # bin b = c*NWIN + w
    wcap = np.array([SUB] * (NWIN - 1) + [DC - SUB * (NWIN - 1)], np.int64)
    cap = np.tile(wcap, NCORES)
    loads = np.zeros((NB, 4), np.float64)
    fill = np.zeros(NB, np.int64)
    target = np.maximum(cnt.sum(0).astype(np.float64), 1.0) / NB

    core_of = np.empty(N, np.int64)
    rel_of = np.empty(N, np.int64)
    pos2node = np.zeros((NCORES, DC), np.int64)
    cnt_f = cnt.astype(np.float64)

    # greedy for nodes with edges; bulk round-robin for isolated nodes
    full = fill >= cap
    for n in order:
        if tot[n] == 0:
            break
        score = ((loads + cnt_f[n]) / target).max(axis=1)
        score[full] = np.inf
        b = int(np.argmin(score))
        loads[b] += cnt_f[n]
        c, w = divmod(b, NWIN)
        core_of[n] = c
        rel_of[n] = w * SUB + fill[b]
        pos2node[c, w * SUB + fill[b]] = n
        fill[b] += 1
        if fill[b] >= cap[b]:
            full[b] = True
    # isolated nodes fill remaining slots
    rest = [n for n in order if tot[n] == 0]
    if rest:
        free_b = np.repeat(np.arange(NB), (cap - fill))
        assert len(free_b) == len(rest)
        for n, b in zip(rest, free_b):
            c, w = divmod(int(b), NWIN)
            core_of[n] = c
            rel_of[n] = w * SUB + fill[b]
            pos2node[c, w * SUB + fill[b]] = n
            fill[b] += 1
    assert (fill == cap).all()
    return core_of, rel_of, pos2node


def _build_schedule(edge_index, edge_attr):
    """Integer-only host preprocessing. Returns (sched, per_core_arrays)."""
    src = np.asarray(edge_index[0]).astype(np.int64)
    dst = np.asarray(edge_index[1]).astype(np.int64)
    attr = np.asarray(edge_attr).astype(np.int64)

    deg = np.stack([
        np.bincount(dst[attr == 1], minlength=N),
        np.bincount(dst[attr == 2], minlength=N),
    ]) + 1  # [2, N] int

    mask = (attr == 1) | (attr == 2)
    s_, d_, k_ = src[mask], dst[mask], attr[mask] - 1  # k_ in {0,1}
    half = (s_ >= LO).astype(np.int64)
    core_of, rel_of, pos2node = _balance_nodes(d_, k_, half)
    core = core_of[d_]
    rel = rel_of[d_]
    sup = rel // SUP
    q_ = (rel % SUP) // SUB

    key = ((((core * NSUP + sup) * 2 + half) * 2 + k_) * NQ + q_)
    order = np.argsort(key, kind="stable")
    ks = key[order]
    NKEY = NCORES * NSUP * 2 * 2 * NQ
    starts = np.searchsorted(ks, np.arange(NKEY + 1))
    cnt = (starts[1:] - starts[:-1]).reshape(NCORES, NSUP, 2, 2, NQ)
    T_run = -(-cnt.max(axis=0) // TILE)  # [NSUP, 2, 2, NQ] shared tile counts

    # ---- uniform tile list ----
    # stream position assignment (gather buffer layout) is per (s, half),
    # but tile PROCESSING order is group-major per (s, k, q): all matmuls of
    # one PSUM accumulation group must be consecutive (2KB zero-region rule).
    tiles = []      # dicts: kind g/self, s, half, k, q, t(stream pos), h
    run_base = np.zeros((NSUP, 2, 2, NQ), np.int64)  # tile offset of run in (s,half) stream
    Tsh = np.zeros((NSUP, 2), np.int64)
    for s in range(NSUP):
        for hf in (0, 1):
            off = 0
            for k in (0, 1):
                for q in range(NQ):
                    run_base[s, hf, k, q] = off
                    off += int(T_run[s, hf, k, q])
            Tsh[s, hf] = off
        for k in (0, 1):
            for q in range(NQ):
                for hf in (0, 1):
                    base = run_base[s, hf, k, q]
                    for j in range(int(T_run[s, hf, k, q])):
                        tiles.append(dict(kind="g", s=s, half=hf, k=k, q=q,
                                          t=int(base) + j))
                for h in (0, 1):
                    if _valid_half(s, q, h):
                        tiles.append(dict(kind="self", s=s, k=k, q=q, h=h))

    # start/stop flags per accumulation group (s, k, q)
    first_seen, last_seen = {}, {}
    for i, t in enumerate(tiles):
        g = (t["s"], t["k"], t["q"])
        if g not in first_seen:
            first_seen[g] = i
        last_seen[g] = i
    for i, t in enumerate(tiles):
        g = (t["s"], t["k"], t["q"])
        t["start"] = first_seen[g] == i
        t["stop"] = last_seen[g] == i

    # gather batches per (s, half): list of (t0, nt), near-equal sizes;
    # super 0 uses small batches so the pipeline ramps quickly
    batches = {}
    for s in range(NSUP):
        for hf in (0, 1):
            gb_s = GB_TILES
            batches[(s, hf)] = _balanced_batches(int(Tsh[s, hf]), gb_s)

    # idx stream column offset (in tiles) per (s, half)
    idx_off = {}
    off = 0
    for s in range(NSUP):
        for hf in (0, 1):
            idx_off[(s, hf)] = off
            off += int(Tsh[s, hf])
    Tg = off  # total gather tiles

    # map (s, half, t) -> global tile idx (meta column)
    gmap = {}
    for i, t in enumerate(tiles):
        if t["kind"] == "g":
            gmap[(t["s"], t["half"], t["t"])] = i
    Ttot = len(tiles)

    # ---- per-core data arrays ----
    per_core = []
    for c in range(NCORES):
        idx_all = np.zeros((128, Tg * 8), np.int16)
        dl = np.full((128, Ttot), -1.0, np.float32)
        m1 = np.ones((128, Ttot), np.float32)

        for s in range(NSUP):
            for hf in (0, 1):
                for k in (0, 1):
                    for q in range(NQ):
                        kk = (((c * NSUP + s) * 2 + hf) * 2 + k) * NQ + q
                        e = order[starts[kk]:starts[kk + 1]]
                        n = len(e)
                        if n == 0:
                            continue
                        g = run_base[s, hf, k, q] * TILE + np.arange(n)
                        # idx layout: logical i -> [i%16 + 16*grp, i//16]
                        iv = (s_[e] - hf * LO).astype(np.int16)
                        gofs = idx_off[(s, hf)] * 8  # column offset (16-col units)
                        for grp in range(8):
                            idx_all[g % 16 + 16 * grp, gofs + g // 16] = iv
                        # meta
                        tloc = g // TILE
                        cols = np.array([gmap[(s, hf, int(t))] for t in tloc])
                        p = g % TILE
                        dl[p, cols] = (rel[e] - s * SUP - q * SUB).astype(np.float32)
                        m1[p, cols] = (deg[k, s_[e]] * deg[k, d_[e]]).astype(np.float32)
        # self tiles
        for i, t in enumerate(tiles):
            if t["kind"] != "self":
                continue
            s, k, q, h = t["s"], t["k"], t["q"], t["h"]
            nrows = _sub_rows(s, q, h)
            p = np.arange(nrows)
            nodes = pos2node[c, s * SUP + q * SUB + h * 128 + p]
            dl[p, i] = (h * 128 + p).astype(np.float32)
            m1[p, i] = (deg[k, nodes] ** 2).astype(np.float32)
        per_core.append(dict(idx=idx_all, dl=dl, m1=m1))

    sched = dict(tiles=tiles, Tsh=Tsh, batches=batches, Ttot=Ttot, gmap=gmap,
                 idx_off=idx_off, Tg=Tg, pos2node=pos2node)
    return sched, per_core


def _build_program(sched, sup_limit=None, skip_fin=False, ncores=NCORES, probe=(), bufs=None):
    from contextlib import ExitStack
    from concourse import bacc, mybir
    import concourse.tile as tile

    f32 = mybir.dt.float32
    f32r = mybir.dt.float32r
    bf16 = mybir.dt.bfloat16
    i16 = mybir.dt.int16
    Alu = mybir.AluOpType
    Act = mybir.ActivationFunctionType

    tiles = sched["tiles"]
    Tsh = sched["Tsh"]
    batches = sched["batches"]
    Ttot = sched["Ttot"]
    idx_off = sched["idx_off"]
    Tg = sched["Tg"]

    bufs = {**dict(oh=10, r=4, ob=6, ac=6, xo=6, gb=6, gbb=12), **(bufs or {})}
    nc = bacc.Bacc("TRN2", target_bir_lowering=False, debug=False,
                   num_devices=ncores, dynamic_dma_scratch_size=SCRATCH,
                   num_swdge_queues=2)

    # DRAM I/O
    x_d = nc.dram_tensor("x", [N, D], f32, kind="ExternalInput").ap()
    xo_d = nc.dram_tensor("xown", [NXO, 128, D], f32, kind="ExternalInput").ap()
    w_d = [nc.dram_tensor(f"W{k+1}", [D, D], f32, kind="ExternalInput").ap()
           for k in (0, 1)]
    b_d = [nc.dram_tensor(f"b{k+1}c", [D, 1], f32, kind="ExternalInput").ap()
           for k in (0, 1)]
    al_d = nc.dram_tensor("alpha2", [1, 2], f32, kind="ExternalInput").ap()
    iota_d = nc.dram_tensor("iota", [128, SUB], f32, kind="ExternalInput").ap()
    iden_d = nc.dram_tensor("ident", [128, 128], f32, kind="ExternalInput").ap()
    ones_d = nc.dram_tensor("ones1", [1, 128], f32, kind="ExternalInput").ap()
    dl_d = nc.dram_tensor("mdl", [128, Ttot], f32, kind="ExternalInput").ap()
    m1_d = nc.dram_tensor("mm1", [128, Ttot], f32, kind="ExternalInput").ap()
    idx_d = nc.dram_tensor("idxall", [128, Tg * 8], i16,
                           kind="ExternalInput").ap()
    out_d = nc.dram_tensor("out", [NWIN, 2, 128, D], bf16,
                           kind="ExternalOutput").ap()

    with tile.TileContext(nc) as tc, ExitStack() as ctx:
        const_p = ctx.enter_context(tc.tile_pool(name="const", bufs=1))
        meta_p = ctx.enter_context(tc.tile_pool(name="meta", bufs=1))
        gb_p = ctx.enter_context(tc.tile_pool(name="gb", bufs=bufs["gb"]))
        if BF16_GB:
            gbb_p = ctx.enter_context(tc.tile_pool(name="gbb", bufs=bufs["gbb"]))
            xob_p = ctx.enter_context(tc.tile_pool(name="xob", bufs=bufs["xo"]))
        oh_p = ctx.enter_context(tc.tile_pool(name="oh", bufs=bufs["oh"]))
        xo_p = ctx.enter_context(tc.tile_pool(name="xo", bufs=bufs["xo"]))
        ac_p = ctx.enter_context(tc.tile_pool(name="ac", bufs=bufs["ac"]))
        r_p = ctx.enter_context(tc.tile_pool(name="r", bufs=bufs["r"]))
        ob_p = ctx.enter_context(tc.tile_pool(name="ob", bufs=bufs["ob"]))
        acc_p = ctx.enter_context(tc.tile_pool(name="acc", bufs=ACC_BUFS, space="PSUM"))
        u_p = ctx.enter_context(tc.tile_pool(name="u", bufs=2, space="PSUM"))
        tp_p = ctx.enter_context(tc.tile_pool(name="tp", bufs=2, space="PSUM"))

        # ---------- prologue ----------
        # idx first: unblocks gather desc-gen immediately
        idx_t = const_p.tile([128, Tg * 8], i16)
        nc.sync.dma_start(idx_t[:], idx_d[:])
        dl_t = meta_p.tile([128, Ttot], f32)
        nc.sync.dma_start(dl_t[:], dl_d[:])
        m1_t = meta_p.tile([128, Ttot], f32)
        nc.sync.dma_start(m1_t[:], m1_d[:])
        iota_t = const_p.tile([128, SUB], f32)
        nc.sync.dma_start(iota_t[:], iota_d[:])
        if BF16_GB:
            iotab_t = const_p.tile([128, SUB], bf16)
            nc.vector.tensor_copy(iotab_t[:], iota_t[:])
        # values = 1/sqrt(m1)
        sq_t = meta_p.tile([128, Ttot], f32)
        nc.scalar.activation(sq_t[:], m1_t[:], Act.Sqrt)
        val_t = meta_p.tile([128, Ttot], f32)
        nc.vector.reciprocal(val_t[:], sq_t[:])

        iden_t = const_p.tile([128, 128], f32r)
        nc.sync.dma_start(iden_t[:], iden_d[:].bitcast(f32r))
        ones_t = const_p.tile([1, 128], f32)
        nc.sync.dma_start(ones_t[:], ones_d[:])
        w_t = []
        for k in (0, 1):
            wt = const_p.tile([128, 128], f32, tag=f"wraw{k}")
            nc.sync.dma_start(wt[:], w_d[k][:])
            w_t.append(wt)
        b_t = []
        for k in (0, 1):
            bt = const_p.tile([128, 1], f32, tag=f"braw{k}")
            nc.sync.dma_start(bt[:], b_d[k][:])
            b_t.append(bt)
        al_t = const_p.tile([1, 2], f32)
        nc.sync.dma_start(al_t[:], al_d[:])

        # softmax(alpha) on device
        e_t = const_p.tile([1, 2], f32)
        nc.scalar.activation(e_t[:], al_t[:], Act.Exp)
        su_t = const_p.tile([1, 1], f32)
        nc.vector.tensor_tensor(su_t[:], e_t[:, 0:1], e_t[:, 1:2], Alu.add)
        rs_t = const_p.tile([1, 1], f32)
        nc.vector.reciprocal(rs_t[:], su_t[:])
        a_t = const_p.tile([1, 2], f32)
        nc.vector.tensor_scalar(a_t[:], e_t[:], rs_t[:], None, Alu.mult)
        # broadcast a over 128 partitions via rank-1 matmul
        abc_ps = u_p.tile([128, SUB], f32, tag="u")
        nc.tensor.matmul(abc_ps[:, 0:2], lhsT=ones_t[:], rhs=a_t[:],
                         start=True, stop=True)
        abc_t = const_p.tile([128, 2], f32)
        nc.vector.tensor_copy(abc_t[:], abc_ps[:, 0:2])
        # W'_k = a_k * W_k ;  bconst = a0*b1 + a1*b2
        wp_t = []
        for k in (0, 1):
            wp = const_p.tile([128, 128], f32, tag=f"wp{k}")
            nc.vector.tensor_scalar(wp[:].bitcast(f32r), w_t[k][:],
                                    abc_t[:, k:k + 1], None, Alu.mult)
            wp_t.append(wp)
        bc0 = const_p.tile([128, 1], f32, tag="btmp0")
        nc.vector.tensor_scalar(bc0[:], b_t[0][:], abc_t[:, 0:1], None, Alu.mult)
        bc1 = const_p.tile([128, 1], f32, tag="btmp1")
        nc.vector.tensor_scalar(bc1[:], b_t[1][:], abc_t[:, 1:2], None, Alu.mult)
        bconst = const_p.tile([128, 1], f32)
        nc.vector.tensor_tensor(bconst[:], bc0[:], bc1[:], Alu.add)

        # ---------- main loop over super-chunks ----------
        oh_rr = 0
        conv_i = 0
        gq = 0
        for s in range(NSUP if sup_limit is None else sup_limit):
            # gather batches for this super-chunk
            gbufs = {}  # (half, batch_index) -> (tile, t0, nt)
            for hf in (0, 1):
                if Tsh[s, hf] == 0:
                    continue
                goff = idx_off[(s, hf)]
                it = idx_t
                for bi, (t0, nt) in enumerate(batches[(s, hf)]):
                    gb = gb_p.tile([128, nt, 128], f32r, tag="gb")
                    src_ap = x_d[0:LO, :] if hf == 0 else x_d[LO:N, :]
                    if "no_gather" not in probe:
                        nc.gpsimd.dma_gather(
                            gb[:], src_ap.bitcast(f32r),
                            it[:, (goff + t0) * 8:(goff + t0 + nt) * 8],
                            nt * 128, nt * 128, 128, queue_num=gq % 2)
                        gq += 1
                    if BF16_GB:
                        gbb = gbb_p.tile([128, nt, 128], bf16, tag="gbb")
                        conv_i += 1
                        if CONV_DVE_MOD and conv_i % CONV_DVE_MOD == 0:
                            nc.vector.tensor_copy(gbb[:], gb[:].bitcast(f32))
                        else:
                            nc.scalar.activation(gbb[:], gb[:].bitcast(f32),
                                                 Act.Copy)
                        gbufs[(hf, bi)] = (gbb, t0, nt)
                    else:
                        gbufs[(hf, bi)] = (gb, t0, nt)

            # x_own tiles for this super-chunk (self matmuls + residual),
            # one batched DMA: [128, 4, 128] <- xo3[s*4:(s+1)*4] transposed
            xo = xo_p.tile([128, XPS, 128], f32r, tag="xo")
            nc.sync.dma_start(xo[:], xo_d[s * XPS:(s + 1) * XPS].transpose([1, 0, 2]).bitcast(f32r))
            if BF16_GB:
                xob = xob_p.tile([128, XPS, 128], bf16, tag="xob")
                if XO_CONV_DVE:
                    nc.vector.tensor_copy(xob[:], xo[:].bitcast(f32))
                else:
                    nc.scalar.activation(xob[:], xo[:].bitcast(f32), Act.Copy)

            def xo_half(q, h):
                return (xob if BF16_GB else xo)[:, q * 2 + h, :]

            # PSUM accumulators [f, SUP] per branch
            accs = [acc_p.tile([128, SUP], f32, tag="acc", name=f"acc{s}_{_k}") for _k in (0, 1)]

            # edge + self tiles
            for i, t in enumerate(tiles):
                if t["s"] != s:
                    continue
                col = i
                if "no_oh" in probe:
                    oh = iota_t
                else:
                    if t["kind"] == "self":
                        eng = nc.gpsimd if SELF_OH_POOL else nc.vector
                    else:
                        eng = (nc.gpsimd
                               if (oh_rr % 100) < int(POOL_OH_FRAC * 100)
                               else nc.vector)
                        oh_rr += 1
                    if BF16_GB:
                        oh = oh_p.tile([128, SUB], bf16, tag="oh")
                        eng.tensor_scalar(oh[:], iotab_t[:],
                                          dl_t[:, col:col + 1],
                                          val_t[:, col:col + 1],
                                          Alu.is_equal, Alu.mult)
                    else:
                        oh = oh_p.tile([128, SUB], f32, tag="oh")
                        eng.tensor_scalar(oh[:].bitcast(f32r), iota_t[:],
                                          dl_t[:, col:col + 1],
                                          val_t[:, col:col + 1],
                                          Alu.is_equal, Alu.mult)
                if t["kind"] == "g":
                    hf = t["half"]
                    # locate batch containing stream pos t
                    for bi, (t0, nt) in enumerate(batches[(s, hf)]):
                        if t0 <= t["t"] < t0 + nt:
                            break
                    gb, t0, nt = gbufs[(hf, bi)]
                    stat = (iden_t[:] if "no_gather" in probe
                            else gb[:, t["t"] - t0, :])
                else:
                    stat = xo_half(t["q"], t["h"])
                q = t["q"]
                if "no_mm" not in probe:
                    rhs_oh = oh[:] if BF16_GB else oh[:].bitcast(f32r)
                    nc.tensor.matmul(accs[t["k"]][:, q * SUB:(q + 1) * SUB],
                                     lhsT=stat, rhs=rhs_oh,
                                     start=t["start"], stop=t["stop"])

            # finalize each 256-wide sub-window
            for q in range(NQ):
                if skip_fin or not _valid_sub(s, q):
                    continue
                u_ps = u_p.tile([128, SUB], f32, tag="u")
                for k in (0, 1):
                    ac = ac_p.tile([128, SUB], f32, tag="ac")
                    if AC_DVE:
                        nc.vector.tensor_copy(ac[:].bitcast(f32r),
                                              accs[k][:, q * SUB:(q + 1) * SUB])
                    else:
                        nc.scalar.activation(ac[:].bitcast(f32r),
                                             accs[k][:, q * SUB:(q + 1) * SUB],
                                             Act.Copy)
                    nc.tensor.matmul(u_ps[:], lhsT=wp_t[k][:].bitcast(f32r),
                                     rhs=ac[:].bitcast(f32r),
                                     start=(k == 0), stop=(k == 1))
                r_t = r_p.tile([128, SUB], f32r, tag="r")
                nc.scalar.activation(r_t[:], u_ps[:], Act.Relu, bias=bconst[:])
                w = s * NQ + q
                ob = ob_p.tile([128, 2, 128], bf16, tag="ob")
                tp2 = tp_p.tile([128, 2, 128], f32, tag="tp")
                nhalf = 1 + _valid_half(s, q, 1)
                for h in range(nhalf):
                    nc.tensor.transpose(tp2[:, h, :].bitcast(f32r),
                                        r_t[:, h * 128:(h + 1) * 128],
                                        iden_t[:])
                res_eng = nc.gpsimd if RES_POOL else nc.vector
                res_eng.tensor_tensor(
                    ob[:, 0:nhalf, :], tp2[:, 0:nhalf, :],
                    xo[:, q * 2:q * 2 + nhalf, :].bitcast(f32), Alu.add)
                if nhalf == 2:
                    nc.sync.dma_start(out_d[w].transpose([1, 0, 2]), ob[:])
                else:
                    nrows = _sub_rows(s, q, 0)
                    nc.sync.dma_start(out_d[w, 0, 0:nrows, :], ob[0:nrows, 0, :])

    nc.compile()
    return nc


def _make_in_maps(x, W1, b1, W2, b2, alpha, sched, per_core):
    pos2node = sched["pos2node"]
    x = np.ascontiguousarray(np.asarray(x, np.float32))
    consts = dict(
        W1=np.asarray(W1, np.float32), W2=np.asarray(W2, np.float32),
        b1c=np.asarray(b1, np.float32).reshape(D, 1),
        b2c=np.asarray(b2, np.float32).reshape(D, 1),
        alpha2=np.asarray(alpha, np.float32).reshape(1, 2),
        iota=np.tile(np.arange(SUB, dtype=np.float32), (128, 1)),
        ident=np.eye(128, dtype=np.float32),
        ones1=np.ones((1, 128), np.float32),
    )
    in_maps = []
    for c in range(NCORES):
        m = dict(consts)
        m["x"] = x
        xop = np.zeros((NXO * 128, D), np.float32)
        xop[:DC] = x[pos2node[c]]
        m["xown"] = xop.reshape(NXO, 128, D)
        m["mdl"] = per_core[c]["dl"]
        m["mm1"] = per_core[c]["m1"]
        m["idxall"] = per_core[c]["idx"]
        in_maps.append(m)
    return in_maps


def _run(inputs, trace=False):
    from concourse.bass_utils import run_bass_kernel_spmd

    sched, per_core = _build_schedule(inputs["edge_index"], inputs["edge_attr"])
    nc = _build_program(sched)
    in_maps = _make_in_maps(inputs["x"], inputs["W1"], inputs["b1"],
                            inputs["W2"], inputs["b2"], inputs["alpha"],
                            sched, per_core)
    res = run_bass_kernel_spmd(nc, in_maps, list(range(NCORES)), trace=trace)
    pos2node = sched["pos2node"]
    out = np.empty((N, D), np.float32)
    for c in range(NCORES):
        o = res.results[c]["out"]  # [NWIN, 2, 128, D]
        out[pos2node[c]] = o.reshape(NWIN * 2 * 128, D)[:DC]
    return out, res


def kernel(x, edge_index, edge_attr, W1, b1, W2, b2, alpha):
    inputs = dict(x=x, edge_index=edge_index, edge_attr=edge_attr,
                  W1=W1, b1=b1, W2=W2, b2=b2, alpha=alpha)
    out, _ = _run(inputs, trace=False)
    return out
